# revision 44
# baseline (speedup 1.0000x reference)
"""Trainium2 Bass kernel for gated-adapter attention (Llama-Adapter style).

Sharding: 2 data-parallel groups of 4 cores (batch dim); within a group the 32
heads are tensor-parallel (8 heads/core).  Each core computes QKV + RoPE +
causal flash-style attention (transposed scores) + gated adapter cross
attention for its heads, AllGathers the per-head attention output across its
group of 4, then computes a column shard of the wo projection.  The host
reassembles the full [2, 2048, 4096] output from the 8 per-core shards.

v2 structure (from trace analysis of v1):
- QKV runs as six half-projection passes (k0,k1,v0,v1,q0,q1), each covering 4
  heads x all 17 token chunks.  Weight sets are 32 tiles of [128,512]
  (32KB/partition), ring-buffered across passes so the PE never waits on a
  weight reload.  x is loaded with ONE line-rate DMA per token chunk (host
  pre-arranges x into [p, tch, m, t] layout) instead of 32 tiny DMAs whose
  issue cost throttled v1.
- Each (pass, tch) is a single 32-matmul accumulation chain into one PSUM
  bank, which lets the hardware hide LDWEIGHTS behind the previous matmul.
- Attention: exp batched over pairs of k-chunks ([128,1024] PSUM tiles); the
  softmax denominator matmul uses a [128,128] ones stationary so the sums come
  out pre-broadcast across partitions (no gpsimd partition_broadcast).
  q-projection runs last so attention overlaps its tail.
- wo: w2 column-half resident (ring prefetched), agout loaded per (jh, qb).

All TensorEngine tensors are fp16; PSUM accumulation is fp32; softmax
sums/reciprocals are fp32.
"""

import math
import os
import sys

import numpy as np

for _p in ("/opt/trn_rl_repo",):
    if os.path.isdir(_p) and _p not in sys.path:
        sys.path.insert(0, _p)

import ml_dtypes  # noqa: E402

import concourse.bass as bass  # noqa: E402
import concourse.mybir as mybir  # noqa: E402
import concourse.tile as tile  # noqa: E402
from concourse import bacc  # noqa: E402

FP16 = np.float16
F16 = mybir.dt.float16
F32 = mybir.dt.float32

DIM = 4096
S = 2048
B = 2
H = 32
HD = 128
ALEN = 10

NCORES = 8
CPG = 4          # cores per group (group = one batch element)
HPC = 8          # heads per core
OC = HPC * HD    # 1024 output dims per core for q/k/v and for wo columns
HH = 4           # heads per half-pass

TCN = 17         # t-chunks of 128: 16 real + 1 adapter/pad chunk
TAUG = TCN * 128  # 2176
NM = DIM // 128   # 32 contraction chunks
QB = 4           # query blocks
QW = 512         # query block width
SCALE = 1.0 / math.sqrt(HD)

REPLICA_GROUPS = [[0, 1, 2, 3], [4, 5, 6, 7]]

Exp = mybir.ActivationFunctionType.Exp
Copy = mybir.ActivationFunctionType.Copy


def _alu():
    from concourse.alu_op_type import AluOpType
    return AluOpType


def build_graph():
    nc = bacc.Bacc(
        "TRN2",
        target_bir_lowering=False,
        debug=False,
        num_devices=NCORES,
    )

    # ---- external I/O ------------------------------------------------------
    # x pre-arranged on host: [128, tch(17), m(32), t(128)] so one DMA per
    # token chunk reads 8KB contiguous per partition.
    xT = nc.dram_tensor("xT", [128, TCN * NM * 128], F16, kind="ExternalInput")
    wqT = nc.dram_tensor("wqT", [DIM, OC], F16, kind="ExternalInput")
    wkT = nc.dram_tensor("wkT", [DIM, OC], F16, kind="ExternalInput")
    wvT = nc.dram_tensor("wvT", [DIM, OC], F16, kind="ExternalInput")
    # wo pre-packed on host: [p(128), ocsub(8), m(32), j(128)]
    w2P = nc.dram_tensor("w2P", [128, 8 * NM * 128], F16, kind="ExternalInput")
    cosP = nc.dram_tensor("cosP", [S, 64], F16, kind="ExternalInput")
    sinP = nc.dram_tensor("sinP", [S, 64], F16, kind="ExternalInput")
    # causal diag-band mask is qb-independent: only 4 distinct tiles
    maskmul = nc.dram_tensor("maskmul", [4, 128, QW], F16, kind="ExternalInput")
    gates = nc.dram_tensor("gates", [16, HPC], F32, kind="ExternalInput")
    eye = nc.dram_tensor("eye", [128, 128], F16, kind="ExternalInput")
    # output transposed per core: [OC, S]; host transposes when unsharding
    out_ext = nc.dram_tensor("out", [OC, S], F32, kind="ExternalOutput")

    op = _alu()
    from contextlib import ExitStack

    with tile.TileContext(nc) as tc:
        with (
            tc.tile_pool(name="glob", bufs=1) as glob,
            tc.tile_pool(name="dram", bufs=1, space="DRAM") as dpool,
        ):
            ones = glob.tile([128, 128], F16, tag="ones")
            negC = glob.tile([128, 1], F32, tag="negC")
            gatesb = glob.tile([16, HPC], F32, tag="gatesb")
            nc.gpsimd.memset(ones[:], 1.0)
            nc.gpsimd.memset(negC[:], -9.0)
            nc.sync.dma_start(gatesb[:], gates[:])

            agin = [dpool.tile([OC, QW], F16, tag=f"agin{q}", name=f"agin{q}")
                    for q in range(QB)]
            # AllGather split in half (heads 0-3 / 4-7) so the collective
            # starts half a block early and drains before wo needs DMA
            agout0 = [dpool.tile([CPG * 512, QW], F16, tag=f"ago0{q}",
                                 name=f"ago0{q}")
                      for q in range(QB)]
            agout1 = [dpool.tile([CPG * 512, QW], F16, tag=f"ago1{q}",
                                 name=f"ago1{q}")
                      for q in range(QB)]

            _esP = ExitStack()
            with _esP:
                persist = _esP.enter_context(
                    tc.tile_pool(name="persist", bufs=1))
                kT = persist.tile([128, HPC * S], F16, tag="kT")     # [d, h*t]
                qT = persist.tile([128, HPC * S], F16, tag="qT")
                vsb = persist.tile([128, TCN * OC], F16, tag="vsb")  # [t,tc*o]
                akT = persist.tile([128, HPC * 16], F16, tag="akT")  # [d,h*16]
                avg = persist.tile([16, HPC * HD], F16, tag="avg")   # [a, h*d]

                # ============ Phase 1: QKV in six half-passes ==============
                _es1 = ExitStack()
                with _es1:
                    P1 = lambda **kw: _es1.enter_context(tc.tile_pool(**kw))
                    cpool = P1(name="consts1", bufs=1)
                    wpool = P1(name="wres", bufs=64)
                    xpool = P1(name="xin", bufs=2)
                    apool = P1(name="asm", bufs=4)
                    rpool = P1(name="rot", bufs=3)
                    pspool = P1(name="ps1", bufs=5, space="PSUM")
                    ptpool = P1(name="pstr", bufs=3, space="PSUM")

                    eyesb = cpool.tile([128, 128], F16, tag="eyesb")
                    cossb = cpool.tile([128, 16 * 64], F16, tag="cossb")
                    sinsb = cpool.tile([128, 16 * 64], F16, tag="sinsb")

                    def emit_consts():
                        nc.sync.dma_start(eyesb[:], eye[:])
                        nc.sync.dma_start(
                            cossb[:].rearrange("p (c d) -> p c d", c=16),
                            cosP[:].rearrange("(c p) d -> p c d", p=128),
                        )
                        nc.sync.dma_start(
                            sinsb[:].rearrange("p (c d) -> p c d", c=16),
                            sinP[:].rearrange("(c p) d -> p c d", p=128),
                        )

                    # (proj, half) passes: k first, q last (so attention can
                    # start on heads 0-3 while q half 1 still runs).
                    passes = [("k", wkT, 0), ("k", wkT, 1),
                              ("v", wvT, 0), ("v", wvT, 1),
                              ("q", wqT, 0), ("q", wqT, 1)]

                    def load_xcol(proj, half, tch):
                        xcol = xpool.tile([128, NM * 128], F16, tag="xin",
                                          name=f"x{proj}{half}_{tch}")
                        # two halves: the chain starts on the first 16
                        # m-chunks while the rest is still in flight
                        base = tch * NM * 128
                        hw = NM * 64
                        nc.sync.dma_start(xcol[:, 0:hw],
                                          xT[:, base:base + hw])
                        nc.sync.dma_start(xcol[:, hw:2 * hw],
                                          xT[:, base + hw:base + 2 * hw])
                        return xcol

                    first_pass = True
                    for proj, wsrc, half in passes:
                        c0 = half * 512  # column offset of this half
                        if proj == "q":
                            tch_list = list(range(16))
                        elif proj == "k":
                            tch_list = list(range(TCN))
                        else:
                            tch_list = [16] + list(range(16))
                        pre_x = {}
                        if first_pass:
                            # first x ahead of all weights; the next two
                            # interleaved so the first chains start early
                            pre_x[tch_list[0]] = load_xcol(proj, half,
                                                           tch_list[0])
                        wres = []
                        for m in range(NM):
                            if first_pass and m in (8, 16):
                                t = tch_list[1 if m == 8 else 2]
                                pre_x[t] = load_xcol(proj, half, t)
                            wt = wpool.tile([128, 512], F16, tag="wres",
                                            name=f"w{proj}{half}_{m}")
                            nc.sync.dma_start(
                                wt[:], wsrc[m * 128:(m + 1) * 128,
                                            c0:c0 + 512])
                            wres.append(wt)
                        if first_pass:
                            emit_consts()
                        first_pass = False
                        for tch in tch_list:
                            xcol = pre_x.pop(tch, None)
                            if xcol is None:
                                xcol = load_xcol(proj, half, tch)
                            ps = pspool.tile([128, 512], F32, tag="ps1",
                                             name=f"ps{proj}{half}_{tch}")
                            for m in range(NM):
                                nc.tensor.matmul(
                                    ps[:],
                                    lhsT=xcol[:, m * 128:(m + 1) * 128],
                                    rhs=wres[m][:],
                                    start=(m == 0),
                                    stop=(m == NM - 1),
                                )
                            if proj == "v":
                                # alternate evac engine to balance load
                                if tch % 2 == 0:
                                    nc.scalar.activation(
                                        vsb[:, tch * OC + c0:
                                            tch * OC + c0 + 512],
                                        ps[:], Copy)
                                else:
                                    nc.vector.tensor_copy(
                                        vsb[:, tch * OC + c0:
                                            tch * OC + c0 + 512],
                                        ps[:])
                                if tch == 16:
                                    for hh in range(HH):
                                        h = half * HH + hh
                                        nc.vector.tensor_scalar(
                                            avg[0:10, h * HD:(h + 1) * HD],
                                            vsb[0:10, 16 * OC + h * HD:
                                                16 * OC + (h + 1) * HD],
                                            gatesb[0:10, h:h + 1],
                                            None,
                                            op.mult,
                                        )
                                continue
                            asmt = apool.tile([128, 512], F16, tag="asm",
                                              name=f"as{proj}{half}_{tch}")
                            if tch % 2 == 0:
                                nc.scalar.activation(asmt[:], ps[:], Copy)
                            else:
                                nc.vector.tensor_copy(asmt[:], ps[:])
                            if tch == 16:
                                # adapter chunk (k only): transpose into akT
                                ptr = ptpool.tile([128, 512], F16, tag="pstr",
                                                  name=f"ptrak{half}")
                                for hh in range(HH):
                                    nc.tensor.transpose(
                                        ptr[:, hh * 128:(hh + 1) * 128],
                                        asmt[:, hh * 128:(hh + 1) * 128],
                                        eyesb[:],
                                    )
                                nc.scalar.activation(
                                    akT[:, half * HH * 16:
                                        (half + 1) * HH * 16].rearrange(
                                            "p (h a) -> p h a", h=HH),
                                    ptr[:].rearrange(
                                        "p (h t) -> p h t",
                                        h=HH)[:, :, 0:16],
                                    Copy,
                                )
                                continue
                            # ---- RoPE (deinterleaved head_dim layout) ----
                            a3 = asmt[:].rearrange(
                                "p (h d) -> p h d", h=HH)[:, :, 0:64]
                            b3 = asmt[:].rearrange(
                                "p (h d) -> p h d", h=HH)[:, :, 64:128]
                            cos1 = cossb[:].rearrange(
                                "p (c o d) -> p c o d", c=16, o=1)[:, tch]
                            sin1 = sinsb[:].rearrange(
                                "p (c o d) -> p c o d", c=16, o=1)[:, tch]
                            cos3, _ = bass.broadcast_tensor_aps(cos1, a3)
                            sin3, _ = bass.broadcast_tensor_aps(sin1, a3)
                            rot = rpool.tile([128, 512], F16, tag="rot",
                                             name=f"rot{proj}{half}_{tch}")
                            ra = rot[:].rearrange(
                                "p (h d) -> p h d", h=HH)[:, :, 0:64]
                            rb = rot[:].rearrange(
                                "p (h d) -> p h d", h=HH)[:, :, 64:128]
                            t1 = rpool.tile([128, HH * 64], F16, tag="rt1",
                                            name=f"rt1{proj}{half}_{tch}")
                            t13 = t1[:].rearrange("p (h d) -> p h d", h=HH)
                            t2 = rpool.tile([128, HH * 64], F16, tag="rt2",
                                            name=f"rt2{proj}{half}_{tch}")
                            t23 = t2[:].rearrange("p (h d) -> p h d", h=HH)
                            nc.vector.tensor_tensor(t13, a3, cos3, op.mult)
                            nc.vector.tensor_tensor(t23, b3, sin3, op.mult)
                            nc.vector.tensor_tensor(ra, t13, t23, op.subtract)
                            nc.vector.tensor_tensor(t13, a3, sin3, op.mult)
                            nc.vector.tensor_tensor(t23, b3, cos3, op.mult)
                            nc.vector.tensor_tensor(rb, t13, t23, op.add)
                            # ---- transpose per head -> kT / qT -----------
                            ptr = ptpool.tile([128, 512], F16, tag="pstr",
                                              name=f"ptr{proj}{half}_{tch}")
                            for hh in range(HH):
                                nc.tensor.transpose(
                                    ptr[:, hh * 128:(hh + 1) * 128],
                                    rot[:, hh * 128:(hh + 1) * 128],
                                    eyesb[:],
                                )
                            dst = kT if proj == "k" else qT
                            dsl = dst[:, half * HH * S:(half + 1) * HH * S]
                            if tch % 2 == 0:
                                nc.scalar.activation(
                                    dsl.rearrange(
                                        "p (h t) -> p h t",
                                        h=HH)[:, :,
                                              tch * 128:(tch + 1) * 128],
                                    ptr[:].rearrange("p (h d) -> p h d",
                                                     h=HH),
                                    Copy,
                                )
                            else:
                                nc.vector.tensor_copy(
                                    dsl.rearrange(
                                        "p (h t) -> p h t",
                                        h=HH)[:, :,
                                              tch * 128:(tch + 1) * 128],
                                    ptr[:].rearrange("p (h d) -> p h d",
                                                     h=HH),
                                )

                # ====== Phase 2/3 container: agout staging spans both ======
                _esW = ExitStack()
                with _esW:
                    agmpool = _esW.enter_context(
                        tc.tile_pool(name="agm", bufs=2))
                    agm = {}

                    # ============ Phase 2: attention / AllGather ===========
                    _es2 = ExitStack()
                    with _es2:
                        P2 = lambda **kw: _es2.enter_context(
                            tc.tile_pool(**kw))
                        prpool = P2(name="probs", bufs=8)
                        mkpool = P2(name="mask", bufs=4)
                        smpool = P2(name="small", bufs=1)
                        rcpool = P2(name="rec", bufs=1)
                        # fp16 pair-sums halve the softmax-denominator
                        # matmul count (one add level: ~5e-4 rel error)
                        pspool2 = P2(name="pairs", bufs=6)
                        # c1/c2 readers are on-engine; c3's reader is the
                        # agin DMA which crawls while an AllGather hogs the
                        # DMA engines -- give c3 a slot per head so pending
                        # writes never backpressure the vector queue
                        ctpool = P2(name="ctmp", bufs=1)
                        pscp = P2(name="psc", bufs=2, space="PSUM")
                        ppvp = P2(name="ppv", bufs=2, space="PSUM")
                        psmp = P2(name="psums", bufs=2, space="PSUM")

                        # causal diag-band masks: 4 tiles, loaded once
                        mtiles = []
                        for dk in range(4):
                            mt = mkpool.tile([128, QW], F16, tag="mask",
                                             name=f"mm{dk}")
                            nc.sync.dma_start(mt[:], maskmul[dk])
                            mtiles.append(mt)

                        def attention_block(qb):
                            kk = (qb + 1) * 4  # causal: k chunks 0..kk-1
                            for h in range(HPC):
                                q_ap = qT[:, h * S + qb * QW:
                                          h * S + (qb + 1) * QW]
                                # sums come out broadcast across partitions
                                sums = psmp.tile([128, QW], F32, tag="sums",
                                                 name=f"sums{qb}_{h}")
                                sumA = psmp.tile([128, QW], F32, tag="sums",
                                                 name=f"sumA{qb}_{h}")
                                pbs = []  # (pb_tile, base_col) per k-chunk
                                pairsums = []
                                for kp in range(kk // 2):
                                    sc = pscp.tile([128, 2 * QW], F32,
                                                   tag="sc",
                                                   name=f"sc{qb}_{h}_{kp}")
                                    for j in range(2):
                                        kc = 2 * kp + j
                                        nc.tensor.matmul(
                                            sc[:, j * QW:(j + 1) * QW],
                                            lhsT=kT[:, h * S + kc * 128:
                                                    h * S + (kc + 1) * 128],
                                            rhs=q_ap,
                                            start=True, stop=True,
                                        )
                                    pb = prpool.tile([128, 2 * QW], F16,
                                                     tag="probs",
                                                     name=f"pb{qb}_{h}_{kp}")
                                    nc.scalar.activation(pb[:], sc[:], Exp,
                                                         bias=negC[:, 0:1],
                                                         scale=SCALE)
                                    for j in range(2):
                                        kc = 2 * kp + j
                                        if kc >= qb * 4:
                                            nc.vector.tensor_tensor(
                                                pb[:, j * QW:(j + 1) * QW],
                                                pb[:, j * QW:(j + 1) * QW],
                                                mtiles[kc - qb * 4][:],
                                                op.mult)
                                        pbs.append((pb, j * QW))
                                    pst = pspool2.tile([128, QW], F16,
                                                       tag="pairs",
                                                       name=f"pr{qb}{h}{kp}")
                                    nc.vector.tensor_tensor(
                                        pst[:], pb[:, 0:QW], pb[:, QW:2 * QW],
                                        op.add)
                                    pairsums.append(pst)
                                pv = ppvp.tile([128, QW], F32, tag="pv",
                                               name=f"pv{qb}_{h}")
                                for i in range(kk):
                                    pb, col = pbs[i]
                                    nc.tensor.matmul(
                                        pv[:],
                                        lhsT=vsb[:, i * OC + h * HD:
                                                 i * OC + (h + 1) * HD],
                                        rhs=pb[:, col:col + QW],
                                        start=(i == 0), stop=(i == kk - 1),
                                    )
                                for kp in range(kk // 2):
                                    nc.tensor.matmul(
                                        sums[:], lhsT=ones[:],
                                        rhs=pairsums[kp][:],
                                        start=(kp == 0),
                                        stop=(kp == kk // 2 - 1),
                                    )
                                # adapter
                                asc = pscp.tile([10, QW], F32, tag="sc",
                                                name=f"asc{qb}_{h}")
                                nc.tensor.matmul(
                                    asc[:], lhsT=akT[:, h * 16:h * 16 + 10],
                                    rhs=q_ap, start=True, stop=True)
                                apb = smpool.tile([10, QW], F16, tag="aprobs",
                                                  name=f"apb{qb}_{h}")
                                nc.scalar.activation(apb[:], asc[:], Exp,
                                                     bias=negC[0:10, 0:1],
                                                     scale=SCALE)
                                nc.tensor.matmul(
                                    sumA[:], lhsT=ones[0:10, :], rhs=apb[:],
                                    start=True, stop=True)
                                apv = ppvp.tile([128, QW], F32, tag="pv",
                                                name=f"apv{qb}_{h}")
                                nc.tensor.matmul(
                                    apv[:],
                                    lhsT=avg[0:10, h * HD:(h + 1) * HD],
                                    rhs=apb[:], start=True, stop=True)
                                # normalize + combine
                                recM = rcpool.tile([128, QW], F32, tag="recM",
                                                   name=f"rM{qb}_{h}")
                                nc.vector.reciprocal_approx_fast(recM[:],
                                                                 sums[:])
                                recA = rcpool.tile([128, QW], F32, tag="recA",
                                                   name=f"rA{qb}_{h}")
                                nc.vector.reciprocal_approx_fast(recA[:],
                                                                 sumA[:])
                                c1 = ctpool.tile([128, QW], F16, tag="c1",
                                                 name=f"c1{qb}_{h}")
                                nc.vector.tensor_tensor(c1[:], pv[:],
                                                        recM[:], op.mult)
                                c2 = ctpool.tile([128, QW], F16, tag="c2",
                                                 name=f"c2{qb}_{h}")
                                nc.vector.tensor_tensor(c2[:], apv[:],
                                                        recA[:], op.mult)
                                c3 = ctpool.tile([128, QW], F16, tag="c3",
                                                 bufs=7,
                                                 name=f"c3{qb}_{h}")
                                nc.vector.tensor_tensor(c3[:], c1[:], c2[:],
                                                        op.add)
                                nc.sync.dma_start(
                                    agin[qb][h * 128:(h + 1) * 128, :],
                                    c3[:])
                                if h == 3:
                                    nc.gpsimd.collective_compute(
                                        "AllGather",
                                        op.bypass,
                                        replica_groups=REPLICA_GROUPS,
                                        ins=[agin[qb][0:512, :].opt()],
                                        outs=[agout0[qb][:].opt()],
                                    )
                                elif h == 7:
                                    nc.gpsimd.collective_compute(
                                        "AllGather",
                                        op.bypass,
                                        replica_groups=REPLICA_GROUPS,
                                        ins=[agin[qb][512:1024, :].opt()],
                                        outs=[agout1[qb][:].opt()],
                                    )
                            # stage the gathered block into SBUF on the
                            # (idle) gpsimd queue so it overlaps the
                            # remaining attention blocks; global dim chunk
                            # m = 8*src_core + head
                            am = agmpool.tile([128, NM * QW], F16, tag="agm",
                                              name=f"agm{qb}")
                            am4 = am[:].rearrange("p (c mm t) -> p c mm t",
                                                  c=CPG, mm=HPC)
                            for cc in range(CPG):
                                nc.gpsimd.dma_start(
                                    am4[:, cc, 0:4, :],
                                    agout0[qb][cc * 512:(cc + 1) * 512,
                                               :].rearrange(
                                        "(mm p) t -> p mm t", p=128),
                                )
                                nc.gpsimd.dma_start(
                                    am4[:, cc, 4:8, :],
                                    agout1[qb][cc * 512:(cc + 1) * 512,
                                               :].rearrange(
                                        "(mm p) t -> p mm t", p=128),
                                )
                            agm[qb] = am

                        for qb in range(QB):
                            attention_block(qb)

                    # ========= Phase 3: wo projection (transposed) =========
                    _es3 = ExitStack()
                    with _es3:
                        P3 = lambda **kw: _es3.enter_context(
                            tc.tile_pool(**kw))
                        w2pool = P3(name="w2s", bufs=4)
                        ostpool = P3(name="ost", bufs=3)
                        pwop = P3(name="pwo", bufs=2, space="PSUM")

                        for qb in range(QB):
                            for ocs in range(8):
                                w2s = w2pool.tile([128, NM * 128], F16,
                                                  tag="w2s",
                                                  name=f"w2s{qb}_{ocs}")
                                # first slabs go via the scalar queue, which
                                # drains before the sync queue's last agin
                                # writes, and chunked so the first chains
                                # stream during AllGather bounce traffic
                                if qb == 0 and ocs < 3:
                                    for cc in range(4):
                                        nc.scalar.dma_start(
                                            w2s[:, cc * 1024:
                                                (cc + 1) * 1024],
                                            w2P[:, ocs * NM * 128 +
                                                cc * 1024:
                                                ocs * NM * 128 +
                                                (cc + 1) * 1024])
                                else:
                                    nc.sync.dma_start(
                                        w2s[:],
                                        w2P[:, ocs * NM * 128:
                                            (ocs + 1) * NM * 128])
                                ps = pwop.tile([128, 512], F32, tag="pwo",
                                               name=f"pwo{qb}{ocs}")
                                for m in range(NM):
                                    nc.tensor.matmul(
                                        ps[:],
                                        lhsT=w2s[:, m * 128:(m + 1) * 128],
                                        rhs=agm[qb][:, m * QW:(m + 1) * QW],
                                        start=(m == 0),
                                        stop=(m == NM - 1),
                                    )
                                st = ostpool.tile([128, 512], F32,
                                                  tag="ost",
                                                  name=f"st{qb}_{ocs}")
                                if ocs % 2 == 0:
                                    nc.scalar.activation(st[:], ps[:], Copy)
                                else:
                                    nc.vector.tensor_copy(st[:], ps[:])
                                nc.sync.dma_start(
                                    out_ext[ocs * 128:(ocs + 1) * 128,
                                            qb * QW:(qb + 1) * QW],
                                    st[:])

    nc.compile()
    return nc


# ---------------------------------------------------------------------------
# host-side input prep + execution
# ---------------------------------------------------------------------------

_DEINT = np.concatenate([np.arange(0, 128, 2), np.arange(1, 128, 2)])


def _prep_inputs(x, adapter, wq, wk, wv, wo, gate, freqs_cos, freqs_sin, mask):
    """Build the per-core input maps."""
    perm = np.concatenate([h * HD + _DEINT for h in range(H)])  # deinterleave
    wqp = wq[perm, :]  # permute output dims of wq/wk for rope layout
    wkp = wk[perm, :]

    # causal diag-band mask tiles are identical for every q-block; build
    # from qb=0
    mm = np.empty((4, 128, QW), FP16)
    for dk in range(4):
        k0 = dk * 128
        mm[dk] = np.exp(mask[0, 0, 0:QW, k0:k0 + 128]).T.astype(FP16)

    xTg = []
    for g in range(B):
        xfull = np.zeros((DIM, TAUG), np.float32)
        xfull[:, :S] = x[g].T
        xfull[:, S:S + ALEN] = adapter[0].T
        # [DIM, TAUG] -> [m(32), p(128), tch(17), t(128)] -> [p, tch, m, t]
        xr = xfull.reshape(NM, 128, TCN, 128).transpose(1, 2, 0, 3)
        xTg.append(np.ascontiguousarray(xr).reshape(128, -1).astype(FP16))

    in_maps = []
    for c in range(NCORES):
        g, ci = divmod(c, CPG)
        osl = slice(ci * OC, (ci + 1) * OC)
        gatesv = np.zeros((16, HPC), np.float32)
        gatesv[:, :] = gate[0, ci * HPC:(ci + 1) * HPC, 0, 0][None, :]
        # wo column shard [DIM, OC] packed as [p, ocsub, m, j]
        woTc = wo[osl].T.astype(FP16)  # [4096, 1024]
        w2p = woTc.reshape(NM, 128, 8, 128).transpose(1, 2, 0, 3)
        in_maps.append({
            "xT": xTg[g],
            "wqT": np.ascontiguousarray(wqp[osl].T).astype(FP16),
            "wkT": np.ascontiguousarray(wkp[osl].T).astype(FP16),
            "wvT": np.ascontiguousarray(wv[osl].T).astype(FP16),
            "w2P": np.ascontiguousarray(w2p).reshape(128, -1),
            "cosP": freqs_cos.astype(FP16),
            "sinP": freqs_sin.astype(FP16),
            "maskmul": mm,
            "gates": gatesv,
            "eye": np.eye(128, dtype=FP16),
        })
    return in_maps


_NC_CACHE = {}
TRACE = bool(int(os.environ.get("BASS_KERNEL_TRACE", "0")))
LAST_EXEC_NS = None
LAST_RESULTS = None


def kernel(x, adapter, wq, wk, wv, wo, gate, freqs_cos, freqs_sin, mask,
           start_pos=0, **_unused):
    global LAST_EXEC_NS, LAST_RESULTS
    from concourse.bass_utils import run_bass_kernel_spmd

    to_np = lambda a: np.asarray(a)
    x, adapter, wq, wk, wv, wo = map(to_np, (x, adapter, wq, wk, wv, wo))
    gate, freqs_cos, freqs_sin, mask = map(
        to_np, (gate, freqs_cos, freqs_sin, mask))

    if "nc" not in _NC_CACHE:
        _NC_CACHE["nc"] = build_graph()
    nc = _NC_CACHE["nc"]

    in_maps = _prep_inputs(x, adapter, wq, wk, wv, wo, gate,
                           freqs_cos, freqs_sin, mask)
    res = run_bass_kernel_spmd(
        nc, in_maps, core_ids=list(range(NCORES)), trace=TRACE)
    LAST_EXEC_NS = res.exec_time_ns
    LAST_RESULTS = res
    out = np.empty((B, S, DIM), np.float32)
    for c in range(NCORES):
        g, ci = divmod(c, CPG)
        out[g, :, ci * OC:(ci + 1) * OC] = res.results[c]["out"].T
    return out


# revision 52
# speedup vs baseline: 1.0146x; 1.0146x over previous
"""Trainium2 Bass kernel for gated-adapter attention (Llama-Adapter style).

Sharding: 2 data-parallel groups of 4 cores (batch dim); within a group the 32
heads are tensor-parallel (8 heads/core).  Each core computes QKV + RoPE +
causal flash-style attention (transposed scores) + gated adapter cross
attention for its heads, AllGathers the per-head attention output across its
group of 4, then computes a column shard of the wo projection.  The host
reassembles the full [2, 2048, 4096] output from the 8 per-core shards.

v2 structure (from trace analysis of v1):
- QKV runs as six half-projection passes (k0,k1,v0,v1,q0,q1), each covering 4
  heads x all 17 token chunks.  Weight sets are 32 tiles of [128,512]
  (32KB/partition), ring-buffered across passes so the PE never waits on a
  weight reload.  x is loaded with ONE line-rate DMA per token chunk (host
  pre-arranges x into [p, tch, m, t] layout) instead of 32 tiny DMAs whose
  issue cost throttled v1.
- Each (pass, tch) is a single 32-matmul accumulation chain into one PSUM
  bank, which lets the hardware hide LDWEIGHTS behind the previous matmul.
- Attention: exp batched over pairs of k-chunks ([128,1024] PSUM tiles); the
  softmax denominator matmul uses a [128,128] ones stationary so the sums come
  out pre-broadcast across partitions (no gpsimd partition_broadcast).
  q-projection runs last so attention overlaps its tail.
- wo: w2 column-half resident (ring prefetched), agout loaded per (jh, qb).

All TensorEngine tensors are fp16; PSUM accumulation is fp32; softmax
sums/reciprocals are fp32.
"""

import math
import os
import sys

import numpy as np

for _p in ("/opt/trn_rl_repo",):
    if os.path.isdir(_p) and _p not in sys.path:
        sys.path.insert(0, _p)

import ml_dtypes  # noqa: E402

import concourse.bass as bass  # noqa: E402
import concourse.mybir as mybir  # noqa: E402
import concourse.tile as tile  # noqa: E402
from concourse import bacc  # noqa: E402

FP16 = np.float16
F16 = mybir.dt.float16
F32 = mybir.dt.float32

DIM = 4096
S = 2048
B = 2
H = 32
HD = 128
ALEN = 10

NCORES = 8
CPG = 4          # cores per group (group = one batch element)
HPC = 8          # heads per core
OC = HPC * HD    # 1024 output dims per core for q/k/v and for wo columns
HH = 4           # heads per half-pass

TCN = 17         # t-chunks of 128: 16 real + 1 adapter/pad chunk
TAUG = TCN * 128  # 2176
NM = DIM // 128   # 32 contraction chunks
QB = 4           # query blocks
QW = 512         # query block width
SCALE = 1.0 / math.sqrt(HD)

REPLICA_GROUPS = [[0, 1, 2, 3], [4, 5, 6, 7]]

Exp = mybir.ActivationFunctionType.Exp
Copy = mybir.ActivationFunctionType.Copy


def _alu():
    from concourse.alu_op_type import AluOpType
    return AluOpType


def build_graph():
    nc = bacc.Bacc(
        "TRN2",
        target_bir_lowering=False,
        debug=False,
        num_devices=NCORES,
    )

    # ---- external I/O ------------------------------------------------------
    # x pre-arranged on host: [128, tch(17), m(32), t(128)] so one DMA per
    # token chunk reads 8KB contiguous per partition.
    xT = nc.dram_tensor("xT", [128, TCN * NM * 128], F16, kind="ExternalInput")
    wqT = nc.dram_tensor("wqT", [DIM, OC], F16, kind="ExternalInput")
    wkT = nc.dram_tensor("wkT", [DIM, OC], F16, kind="ExternalInput")
    wvT = nc.dram_tensor("wvT", [DIM, OC], F16, kind="ExternalInput")
    # wo pre-packed on host: [p(128), ocsub(8), m(32), j(128)]
    w2P = nc.dram_tensor("w2P", [128, 8 * NM * 128], F16, kind="ExternalInput")
    cosP = nc.dram_tensor("cosP", [S, 64], F16, kind="ExternalInput")
    sinP = nc.dram_tensor("sinP", [S, 64], F16, kind="ExternalInput")
    # causal diag-band mask is qb-independent: only 4 distinct tiles
    maskmul = nc.dram_tensor("maskmul", [4, 128, QW], F16, kind="ExternalInput")
    gates = nc.dram_tensor("gates", [16, HPC], F32, kind="ExternalInput")
    eye = nc.dram_tensor("eye", [128, 128], F16, kind="ExternalInput")
    # output transposed per core: [OC, S]; host transposes when unsharding
    out_ext = nc.dram_tensor("out", [OC, S], F32, kind="ExternalOutput")

    op = _alu()
    from contextlib import ExitStack

    with tile.TileContext(nc) as tc:
        with (
            tc.tile_pool(name="glob", bufs=1) as glob,
            tc.tile_pool(name="dram", bufs=1, space="DRAM") as dpool,
        ):
            ones = glob.tile([128, 128], F16, tag="ones")
            negC = glob.tile([128, 1], F32, tag="negC")
            gatesb = glob.tile([16, HPC], F32, tag="gatesb")
            nc.gpsimd.memset(ones[:], 1.0)
            nc.gpsimd.memset(negC[:], -9.0)
            nc.sync.dma_start(gatesb[:], gates[:])

            agin = [dpool.tile([OC, QW], F16, tag=f"agin{q}", name=f"agin{q}")
                    for q in range(QB)]
            # AllGather split in half (heads 0-3 / 4-7) so the collective
            # starts half a block early and drains before wo needs DMA
            agout0 = [dpool.tile([CPG * 512, QW], F16, tag=f"ago0{q}",
                                 name=f"ago0{q}")
                      for q in range(QB)]
            agout1 = [dpool.tile([CPG * 512, QW], F16, tag=f"ago1{q}",
                                 name=f"ago1{q}")
                      for q in range(QB)]

            _esP = ExitStack()
            with _esP:
                persist = _esP.enter_context(
                    tc.tile_pool(name="persist", bufs=1))
                kT = persist.tile([128, HPC * S], F16, tag="kT")     # [d, h*t]
                qT = persist.tile([128, HPC * S], F16, tag="qT")
                vsb = persist.tile([128, TCN * OC], F16, tag="vsb")  # [t,tc*o]
                akT = persist.tile([128, HPC * 16], F16, tag="akT")  # [d,h*16]
                avg = persist.tile([16, HPC * HD], F16, tag="avg")   # [a, h*d]

                # ============ Phase 1: QKV in six half-passes ==============
                _es1 = ExitStack()
                with _es1:
                    P1 = lambda **kw: _es1.enter_context(tc.tile_pool(**kw))
                    cpool = P1(name="consts1", bufs=1)
                    wpool = P1(name="wres", bufs=64)
                    xpool = P1(name="xin", bufs=2)
                    apool = P1(name="asm", bufs=4)
                    rpool = P1(name="rot", bufs=3)
                    pspool = P1(name="ps1", bufs=5, space="PSUM")
                    ptpool = P1(name="pstr", bufs=3, space="PSUM")

                    eyesb = cpool.tile([128, 128], F16, tag="eyesb")
                    cossb = cpool.tile([128, 16 * 64], F16, tag="cossb")
                    sinsb = cpool.tile([128, 16 * 64], F16, tag="sinsb")

                    def emit_consts():
                        nc.sync.dma_start(eyesb[:], eye[:])
                        nc.sync.dma_start(
                            cossb[:].rearrange("p (c d) -> p c d", c=16),
                            cosP[:].rearrange("(c p) d -> p c d", p=128),
                        )
                        nc.sync.dma_start(
                            sinsb[:].rearrange("p (c d) -> p c d", c=16),
                            sinP[:].rearrange("(c p) d -> p c d", p=128),
                        )

                    # (proj, half) passes: k first, q last (so attention can
                    # start on heads 0-3 while q half 1 still runs).
                    passes = [("k", wkT, 0), ("k", wkT, 1),
                              ("v", wvT, 0), ("v", wvT, 1),
                              ("q", wqT, 0), ("q", wqT, 1)]

                    def load_xcol(proj, half, tch):
                        xcol = xpool.tile([128, NM * 128], F16, tag="xin",
                                          name=f"x{proj}{half}_{tch}")
                        nc.sync.dma_start(
                            xcol[:],
                            xT[:, tch * NM * 128:(tch + 1) * NM * 128])
                        return xcol

                    first_pass = True
                    for proj, wsrc, half in passes:
                        c0 = half * 512  # column offset of this half
                        if proj == "q":
                            tch_list = list(range(16))
                        elif proj == "k":
                            tch_list = list(range(TCN))
                        else:
                            tch_list = [16] + list(range(16))
                        pre_x = {}
                        if first_pass:
                            # first x ahead of all weights; the next two
                            # interleaved so the first chains start early
                            pre_x[tch_list[0]] = load_xcol(proj, half,
                                                           tch_list[0])
                        wres = []
                        for m in range(NM):
                            if first_pass and m in (8, 16):
                                t = tch_list[1 if m == 8 else 2]
                                pre_x[t] = load_xcol(proj, half, t)
                            wt = wpool.tile([128, 512], F16, tag="wres",
                                            name=f"w{proj}{half}_{m}")
                            nc.sync.dma_start(
                                wt[:], wsrc[m * 128:(m + 1) * 128,
                                            c0:c0 + 512])
                            wres.append(wt)
                        if first_pass:
                            emit_consts()
                        first_pass = False
                        for tch in tch_list:
                            xcol = pre_x.pop(tch, None)
                            if xcol is None:
                                xcol = load_xcol(proj, half, tch)
                            ps = pspool.tile([128, 512], F32, tag="ps1",
                                             name=f"ps{proj}{half}_{tch}")
                            for m in range(NM):
                                nc.tensor.matmul(
                                    ps[:],
                                    lhsT=xcol[:, m * 128:(m + 1) * 128],
                                    rhs=wres[m][:],
                                    start=(m == 0),
                                    stop=(m == NM - 1),
                                )
                            if proj == "v":
                                # alternate evac engine to balance load
                                if tch % 2 == 0:
                                    nc.scalar.activation(
                                        vsb[:, tch * OC + c0:
                                            tch * OC + c0 + 512],
                                        ps[:], Copy)
                                else:
                                    nc.vector.tensor_copy(
                                        vsb[:, tch * OC + c0:
                                            tch * OC + c0 + 512],
                                        ps[:])
                                if tch == 16:
                                    for hh in range(HH):
                                        h = half * HH + hh
                                        nc.vector.tensor_scalar(
                                            avg[0:10, h * HD:(h + 1) * HD],
                                            vsb[0:10, 16 * OC + h * HD:
                                                16 * OC + (h + 1) * HD],
                                            gatesb[0:10, h:h + 1],
                                            None,
                                            op.mult,
                                        )
                                continue
                            asmt = apool.tile([128, 512], F16, tag="asm",
                                              name=f"as{proj}{half}_{tch}")
                            if tch % 2 == 0:
                                nc.scalar.activation(asmt[:], ps[:], Copy)
                            else:
                                nc.vector.tensor_copy(asmt[:], ps[:])
                            if tch == 16:
                                # adapter chunk (k only): transpose into akT
                                ptr = ptpool.tile([128, 512], F16, tag="pstr",
                                                  name=f"ptrak{half}")
                                for hh in range(HH):
                                    nc.tensor.transpose(
                                        ptr[:, hh * 128:(hh + 1) * 128],
                                        asmt[:, hh * 128:(hh + 1) * 128],
                                        eyesb[:],
                                    )
                                nc.scalar.activation(
                                    akT[:, half * HH * 16:
                                        (half + 1) * HH * 16].rearrange(
                                            "p (h a) -> p h a", h=HH),
                                    ptr[:].rearrange(
                                        "p (h t) -> p h t",
                                        h=HH)[:, :, 0:16],
                                    Copy,
                                )
                                continue
                            # ---- RoPE (deinterleaved head_dim layout) ----
                            a3 = asmt[:].rearrange(
                                "p (h d) -> p h d", h=HH)[:, :, 0:64]
                            b3 = asmt[:].rearrange(
                                "p (h d) -> p h d", h=HH)[:, :, 64:128]
                            cos1 = cossb[:].rearrange(
                                "p (c o d) -> p c o d", c=16, o=1)[:, tch]
                            sin1 = sinsb[:].rearrange(
                                "p (c o d) -> p c o d", c=16, o=1)[:, tch]
                            cos3, _ = bass.broadcast_tensor_aps(cos1, a3)
                            sin3, _ = bass.broadcast_tensor_aps(sin1, a3)
                            rot = rpool.tile([128, 512], F16, tag="rot",
                                             name=f"rot{proj}{half}_{tch}")
                            ra = rot[:].rearrange(
                                "p (h d) -> p h d", h=HH)[:, :, 0:64]
                            rb = rot[:].rearrange(
                                "p (h d) -> p h d", h=HH)[:, :, 64:128]
                            t1 = rpool.tile([128, HH * 64], F16, tag="rt1",
                                            name=f"rt1{proj}{half}_{tch}")
                            t13 = t1[:].rearrange("p (h d) -> p h d", h=HH)
                            t2 = rpool.tile([128, HH * 64], F16, tag="rt2",
                                            name=f"rt2{proj}{half}_{tch}")
                            t23 = t2[:].rearrange("p (h d) -> p h d", h=HH)
                            nc.vector.tensor_tensor(t13, a3, cos3, op.mult)
                            nc.vector.tensor_tensor(t23, b3, sin3, op.mult)
                            nc.vector.tensor_tensor(ra, t13, t23, op.subtract)
                            nc.vector.tensor_tensor(t13, a3, sin3, op.mult)
                            nc.vector.tensor_tensor(t23, b3, cos3, op.mult)
                            nc.vector.tensor_tensor(rb, t13, t23, op.add)
                            # ---- transpose per head -> kT / qT -----------
                            ptr = ptpool.tile([128, 512], F16, tag="pstr",
                                              name=f"ptr{proj}{half}_{tch}")
                            for hh in range(HH):
                                nc.tensor.transpose(
                                    ptr[:, hh * 128:(hh + 1) * 128],
                                    rot[:, hh * 128:(hh + 1) * 128],
                                    eyesb[:],
                                )
                            dst = kT if proj == "k" else qT
                            dsl = dst[:, half * HH * S:(half + 1) * HH * S]
                            if tch % 2 == 0:
                                nc.scalar.activation(
                                    dsl.rearrange(
                                        "p (h t) -> p h t",
                                        h=HH)[:, :,
                                              tch * 128:(tch + 1) * 128],
                                    ptr[:].rearrange("p (h d) -> p h d",
                                                     h=HH),
                                    Copy,
                                )
                            else:
                                nc.vector.tensor_copy(
                                    dsl.rearrange(
                                        "p (h t) -> p h t",
                                        h=HH)[:, :,
                                              tch * 128:(tch + 1) * 128],
                                    ptr[:].rearrange("p (h d) -> p h d",
                                                     h=HH),
                                )

                # ====== Phase 2/3 container: agout staging spans both ======
                _esW = ExitStack()
                with _esW:
                    agmpool = _esW.enter_context(
                        tc.tile_pool(name="agm", bufs=2))
                    agm = {}

                    # ============ Phase 2: attention / AllGather ===========
                    _es2 = ExitStack()
                    with _es2:
                        P2 = lambda **kw: _es2.enter_context(
                            tc.tile_pool(**kw))
                        prpool = P2(name="probs", bufs=8)
                        mkpool = P2(name="mask", bufs=4)
                        smpool = P2(name="small", bufs=1)
                        rcpool = P2(name="rec", bufs=1)
                        # fp16 pair-sums halve the softmax-denominator
                        # matmul count (one add level: ~5e-4 rel error)
                        pspool2 = P2(name="pairs", bufs=6)
                        # c1/c2 readers are on-engine; c3's reader is the
                        # agin DMA which crawls while an AllGather hogs the
                        # DMA engines -- give c3 a slot per head so pending
                        # writes never backpressure the vector queue
                        ctpool = P2(name="ctmp", bufs=1)
                        pscp = P2(name="psc", bufs=2, space="PSUM")
                        ppvp = P2(name="ppv", bufs=2, space="PSUM")
                        psmp = P2(name="psums", bufs=2, space="PSUM")

                        # causal diag-band masks: 4 tiles, loaded once
                        mtiles = []
                        for dk in range(4):
                            mt = mkpool.tile([128, QW], F16, tag="mask",
                                             name=f"mm{dk}")
                            nc.sync.dma_start(mt[:], maskmul[dk])
                            mtiles.append(mt)

                        def attention_block(qb):
                            kk = (qb + 1) * 4  # causal: k chunks 0..kk-1
                            for h in range(HPC):
                                q_ap = qT[:, h * S + qb * QW:
                                          h * S + (qb + 1) * QW]
                                # sums come out broadcast across partitions
                                sums = psmp.tile([128, QW], F32, tag="sums",
                                                 name=f"sums{qb}_{h}")
                                sumA = psmp.tile([128, QW], F32, tag="sums",
                                                 name=f"sumA{qb}_{h}")
                                pbs = []  # (pb_tile, base_col) per k-chunk
                                pairsums = []
                                for kp in range(kk // 2):
                                    sc = pscp.tile([128, 2 * QW], F32,
                                                   tag="sc",
                                                   name=f"sc{qb}_{h}_{kp}")
                                    for j in range(2):
                                        kc = 2 * kp + j
                                        # qb0/h0 writes full width so the
                                        # score ring slots always hold finite
                                        # values; every later use skips
                                        # columns below the causal boundary
                                        # (the stale remainder exps to finite
                                        # garbage that the mask zeroes)
                                        ql = max(0, (kc - qb * 4) * 128) \
                                            if (qb > 0 or h > 0) else 0
                                        nc.tensor.matmul(
                                            sc[:, j * QW + ql:
                                               (j + 1) * QW],
                                            lhsT=kT[:, h * S + kc * 128:
                                                    h * S + (kc + 1) * 128],
                                            rhs=q_ap[:, ql:QW],
                                            start=True, stop=True,
                                        )
                                    pb = prpool.tile([128, 2 * QW], F16,
                                                     tag="probs",
                                                     name=f"pb{qb}_{h}_{kp}")
                                    # skip dead leading columns; qb0's full
                                    # exps initialize all probs slots
                                    el = max(0, (2 * kp - qb * 4) * 128) \
                                        if qb > 0 else 0
                                    nc.scalar.activation(pb[:, el:],
                                                         sc[:, el:], Exp,
                                                         bias=negC[:, 0:1],
                                                         scale=SCALE)
                                    for j in range(2):
                                        kc = 2 * kp + j
                                        if kc >= qb * 4:
                                            nc.vector.tensor_tensor(
                                                pb[:, j * QW:(j + 1) * QW],
                                                pb[:, j * QW:(j + 1) * QW],
                                                mtiles[kc - qb * 4][:],
                                                op.mult)
                                        pbs.append((pb, j * QW))
                                    pst = pspool2.tile([128, QW], F16,
                                                       tag="pairs",
                                                       name=f"pr{qb}{h}{kp}")
                                    nc.vector.tensor_tensor(
                                        pst[:], pb[:, 0:QW], pb[:, QW:2 * QW],
                                        op.add)
                                    pairsums.append(pst)
                                pv = ppvp.tile([128, QW], F32, tag="pv",
                                               name=f"pv{qb}_{h}")
                                for i in range(kk):
                                    pb, col = pbs[i]
                                    # diagonal chunks: columns below the
                                    # causal boundary are zero (masked) --
                                    # skip streaming them.  chunk 0 is always
                                    # full so start=True sets has_written
                                    # across the bank.
                                    qlo = max(0, (i - qb * 4) * 128)
                                    nc.tensor.matmul(
                                        pv[:, qlo:QW],
                                        lhsT=vsb[:, i * OC + h * HD:
                                                 i * OC + (h + 1) * HD],
                                        rhs=pb[:, col + qlo:col + QW],
                                        start=(i == 0), stop=(i == kk - 1),
                                    )
                                for kp in range(kk // 2):
                                    qlo = max(0, (2 * kp - qb * 4) * 128)
                                    nc.tensor.matmul(
                                        sums[:, qlo:QW], lhsT=ones[:],
                                        rhs=pairsums[kp][:, qlo:QW],
                                        start=(kp == 0),
                                        stop=(kp == kk // 2 - 1),
                                    )
                                # adapter
                                asc = pscp.tile([10, QW], F32, tag="sc",
                                                name=f"asc{qb}_{h}")
                                nc.tensor.matmul(
                                    asc[:], lhsT=akT[:, h * 16:h * 16 + 10],
                                    rhs=q_ap, start=True, stop=True)
                                apb = smpool.tile([10, QW], F16, tag="aprobs",
                                                  name=f"apb{qb}_{h}")
                                nc.scalar.activation(apb[:], asc[:], Exp,
                                                     bias=negC[0:10, 0:1],
                                                     scale=SCALE)
                                nc.tensor.matmul(
                                    sumA[:], lhsT=ones[0:10, :], rhs=apb[:],
                                    start=True, stop=True)
                                apv = ppvp.tile([128, QW], F32, tag="pv",
                                                name=f"apv{qb}_{h}")
                                nc.tensor.matmul(
                                    apv[:],
                                    lhsT=avg[0:10, h * HD:(h + 1) * HD],
                                    rhs=apb[:], start=True, stop=True)
                                # normalize + combine
                                recM = rcpool.tile([128, QW], F32, tag="recM",
                                                   name=f"rM{qb}_{h}")
                                nc.vector.reciprocal_approx_fast(recM[:],
                                                                 sums[:])
                                recA = rcpool.tile([128, QW], F32, tag="recA",
                                                   name=f"rA{qb}_{h}")
                                nc.vector.reciprocal_approx_fast(recA[:],
                                                                 sumA[:])
                                c1 = ctpool.tile([128, QW], F16, tag="c1",
                                                 name=f"c1{qb}_{h}")
                                nc.vector.tensor_tensor(c1[:], pv[:],
                                                        recM[:], op.mult)
                                c2 = ctpool.tile([128, QW], F16, tag="c2",
                                                 name=f"c2{qb}_{h}")
                                nc.vector.tensor_tensor(c2[:], apv[:],
                                                        recA[:], op.mult)
                                c3 = ctpool.tile([128, QW], F16, tag="c3",
                                                 bufs=7,
                                                 name=f"c3{qb}_{h}")
                                nc.vector.tensor_tensor(c3[:], c1[:], c2[:],
                                                        op.add)
                                nc.sync.dma_start(
                                    agin[qb][h * 128:(h + 1) * 128, :],
                                    c3[:])
                                if h == 3:
                                    nc.gpsimd.collective_compute(
                                        "AllGather",
                                        op.bypass,
                                        replica_groups=REPLICA_GROUPS,
                                        ins=[agin[qb][0:512, :].opt()],
                                        outs=[agout0[qb][:].opt()],
                                    )
                                elif h == 7:
                                    nc.gpsimd.collective_compute(
                                        "AllGather",
                                        op.bypass,
                                        replica_groups=REPLICA_GROUPS,
                                        ins=[agin[qb][512:1024, :].opt()],
                                        outs=[agout1[qb][:].opt()],
                                    )
                            # stage the gathered block into SBUF on the
                            # (idle) gpsimd queue so it overlaps the
                            # remaining attention blocks; global dim chunk
                            # m = 8*src_core + head
                            am = agmpool.tile([128, NM * QW], F16, tag="agm",
                                              name=f"agm{qb}")
                            am4 = am[:].rearrange("p (c mm t) -> p c mm t",
                                                  c=CPG, mm=HPC)
                            for cc in range(CPG):
                                nc.gpsimd.dma_start(
                                    am4[:, cc, 0:4, :],
                                    agout0[qb][cc * 512:(cc + 1) * 512,
                                               :].rearrange(
                                        "(mm p) t -> p mm t", p=128),
                                )
                                nc.gpsimd.dma_start(
                                    am4[:, cc, 4:8, :],
                                    agout1[qb][cc * 512:(cc + 1) * 512,
                                               :].rearrange(
                                        "(mm p) t -> p mm t", p=128),
                                )
                            agm[qb] = am

                        for qb in range(QB):
                            attention_block(qb)

                    # ========= Phase 3: wo projection (transposed) =========
                    _es3 = ExitStack()
                    with _es3:
                        P3 = lambda **kw: _es3.enter_context(
                            tc.tile_pool(**kw))
                        w2pool = P3(name="w2s", bufs=3)
                        ostpool = P3(name="ost", bufs=3)
                        pwop = P3(name="pwo", bufs=2, space="PSUM")

                        for qb in range(QB):
                            for ocs in range(8):
                                w2s = w2pool.tile([128, NM * 128], F16,
                                                  tag="w2s",
                                                  name=f"w2s{qb}_{ocs}")
                                # first slabs go via the scalar queue, which
                                # drains before the sync queue's last agin
                                # writes, and chunked so the first chains
                                # stream during AllGather bounce traffic
                                if qb == 0 and ocs < 2:
                                    for cc in range(4):
                                        nc.scalar.dma_start(
                                            w2s[:, cc * 1024:
                                                (cc + 1) * 1024],
                                            w2P[:, ocs * NM * 128 +
                                                cc * 1024:
                                                ocs * NM * 128 +
                                                (cc + 1) * 1024])
                                else:
                                    nc.sync.dma_start(
                                        w2s[:],
                                        w2P[:, ocs * NM * 128:
                                            (ocs + 1) * NM * 128])
                                ps = pwop.tile([128, 512], F32, tag="pwo",
                                               name=f"pwo{qb}{ocs}")
                                for m in range(NM):
                                    nc.tensor.matmul(
                                        ps[:],
                                        lhsT=w2s[:, m * 128:(m + 1) * 128],
                                        rhs=agm[qb][:, m * QW:(m + 1) * QW],
                                        start=(m == 0),
                                        stop=(m == NM - 1),
                                    )
                                st = ostpool.tile([128, 512], F32,
                                                  tag="ost",
                                                  name=f"st{qb}_{ocs}")
                                if ocs % 2 == 0:
                                    nc.scalar.activation(st[:], ps[:], Copy)
                                else:
                                    nc.vector.tensor_copy(st[:], ps[:])
                                nc.sync.dma_start(
                                    out_ext[ocs * 128:(ocs + 1) * 128,
                                            qb * QW:(qb + 1) * QW],
                                    st[:])

    nc.compile()
    return nc


# ---------------------------------------------------------------------------
# host-side input prep + execution
# ---------------------------------------------------------------------------

_DEINT = np.concatenate([np.arange(0, 128, 2), np.arange(1, 128, 2)])


def _prep_inputs(x, adapter, wq, wk, wv, wo, gate, freqs_cos, freqs_sin, mask):
    """Build the per-core input maps."""
    perm = np.concatenate([h * HD + _DEINT for h in range(H)])  # deinterleave
    wqp = wq[perm, :]  # permute output dims of wq/wk for rope layout
    wkp = wk[perm, :]

    # causal diag-band mask tiles are identical for every q-block; build
    # from qb=0
    mm = np.empty((4, 128, QW), FP16)
    for dk in range(4):
        k0 = dk * 128
        mm[dk] = np.exp(mask[0, 0, 0:QW, k0:k0 + 128]).T.astype(FP16)

    xTg = []
    for g in range(B):
        xfull = np.zeros((DIM, TAUG), np.float32)
        xfull[:, :S] = x[g].T
        xfull[:, S:S + ALEN] = adapter[0].T
        # [DIM, TAUG] -> [m(32), p(128), tch(17), t(128)] -> [p, tch, m, t]
        xr = xfull.reshape(NM, 128, TCN, 128).transpose(1, 2, 0, 3)
        xTg.append(np.ascontiguousarray(xr).reshape(128, -1).astype(FP16))

    in_maps = []
    for c in range(NCORES):
        g, ci = divmod(c, CPG)
        osl = slice(ci * OC, (ci + 1) * OC)
        gatesv = np.zeros((16, HPC), np.float32)
        gatesv[:, :] = gate[0, ci * HPC:(ci + 1) * HPC, 0, 0][None, :]
        # wo column shard [DIM, OC] packed as [p, ocsub, m, j]
        woTc = wo[osl].T.astype(FP16)  # [4096, 1024]
        w2p = woTc.reshape(NM, 128, 8, 128).transpose(1, 2, 0, 3)
        in_maps.append({
            "xT": xTg[g],
            "wqT": np.ascontiguousarray(wqp[osl].T).astype(FP16),
            "wkT": np.ascontiguousarray(wkp[osl].T).astype(FP16),
            "wvT": np.ascontiguousarray(wv[osl].T).astype(FP16),
            "w2P": np.ascontiguousarray(w2p).reshape(128, -1),
            "cosP": freqs_cos.astype(FP16),
            "sinP": freqs_sin.astype(FP16),
            "maskmul": mm,
            "gates": gatesv,
            "eye": np.eye(128, dtype=FP16),
        })
    return in_maps


_NC_CACHE = {}
TRACE = bool(int(os.environ.get("BASS_KERNEL_TRACE", "0")))
LAST_EXEC_NS = None
LAST_RESULTS = None


def kernel(x, adapter, wq, wk, wv, wo, gate, freqs_cos, freqs_sin, mask,
           start_pos=0, **_unused):
    global LAST_EXEC_NS, LAST_RESULTS
    from concourse.bass_utils import run_bass_kernel_spmd

    to_np = lambda a: np.asarray(a)
    x, adapter, wq, wk, wv, wo = map(to_np, (x, adapter, wq, wk, wv, wo))
    gate, freqs_cos, freqs_sin, mask = map(
        to_np, (gate, freqs_cos, freqs_sin, mask))

    if "nc" not in _NC_CACHE:
        _NC_CACHE["nc"] = build_graph()
    nc = _NC_CACHE["nc"]

    in_maps = _prep_inputs(x, adapter, wq, wk, wv, wo, gate,
                           freqs_cos, freqs_sin, mask)
    res = run_bass_kernel_spmd(
        nc, in_maps, core_ids=list(range(NCORES)), trace=TRACE)
    LAST_EXEC_NS = res.exec_time_ns
    LAST_RESULTS = res
    out = np.empty((B, S, DIM), np.float32)
    for c in range(NCORES):
        g, ci = divmod(c, CPG)
        out[g, :, ci * OC:(ci + 1) * OC] = res.results[c]["out"].T
    return out
